# revision 12
# baseline (speedup 1.0000x reference)
"""Trainium2 Bass kernel for nn_AttenConv1d (GNN message passing attention).

Per node n (batch b):
  x_i = x[b, idx1[n,:]]   [16,128]   (centers)
  x_j = x[b, idx0[n,:]]   [16,128]   (neighbors)
  S = x_i @ x_j.T / sqrt(128)        [16,16]
  P = softmax(S, -1)
  h = (P @ x_j).sum(0)               [128]
  y = relu((x[b,n] + h) @ W.T + b)

8 cores: core c handles batch c//4, node slice (c%4)*4096. This problem is
tunnel-transfer-bound (axon H2D ~68MB/s, D2H ~29MB/s), so the pipeline is
built to minimize host<->device bytes:
  - x ships once as bf16 node shards (1MB/core); a jax prelude all-gathers
    the full 32768-row two-batch table on-device and feeds it directly as
    the bass kernel's table parameter (device-resident, no replication over
    the tunnel). mask/b1/W^T/bias-broadcast consts are also built on-device.
  - indices ship un-replicated as [16, nch, 264] i16 and are broadcast to
    the 128-partition wrapped layout dma_gather wants with 8 on-device DMAs.
    Layout per chunk of 128 nodes: [XI 2048 | XJ 2048 | OWN 128] tokens;
    the OWN block doubles as the residual (replaces a PE transpose of a
    separately-shipped xown tensor), and the XJ block doubles as the row
    gather index list (replaces a separate idxr tensor).
  - y returns as bf16 (halves the slow D2H) and is upcast on host.
On-chip per core: bf16 table [128, 256, 128] in SBUF; dma_gather(transpose)
for score columns, DRAM row gather for values; groups of 8 nodes = 128
(node,k) pairs fill the partition dim; block-diagonal bf16 score matmul,
masked exp softmax with fused row-sum, two small matmuls per group, fused
final linear.
"""

import concurrent.futures as cf
import math
import sys

import numpy as np

for _p in ("/opt/trn_rl_repo",):
    if _p not in sys.path:
        sys.path.insert(0, _p)

import jax
import jax.numpy as jnp
import ml_dtypes
from jax.sharding import Mesh, PartitionSpec as P

try:
    from jax.experimental.shard_map import shard_map
except ImportError:
    from jax.shard_map import shard_map

import concourse.bass as bass
import concourse.bacc as bacc
import concourse.mybir as mybir
from concourse import bass2jax, library_config, tile

B, N, K, C = 2, 16384, 128 // 8, 128  # K=16
CORES = 8
TOTN = B * N                  # 32768 rows in the fused two-batch table
NPC = TOTN // CORES           # nodes per core = 4096
CHUNK = 128                   # nodes per chunk
NCH = NPC // CHUNK            # chunks per core = 32
G = 16                        # groups per chunk (8 nodes each)
GN = CHUNK // G               # nodes per group = 8
NTOK = 2 * CHUNK * G + CHUNK  # gathered col tokens per chunk = 4224
SCALE = 1.0 / math.sqrt(C)

f32 = mybir.dt.float32
bf16 = mybir.dt.bfloat16
i16 = mybir.dt.int16


def build_nc():
    nc = bacc.Bacc("TRN2", target_bir_lowering=False, debug=False)
    xtab = nc.dram_tensor("xtab", [TOTN, C], bf16, kind="ExternalInput").ap()
    idxw = nc.dram_tensor("idxw", [16, NCH, NTOK // 16], i16, kind="ExternalInput").ap()
    maskneg = nc.dram_tensor("maskneg", [128, 128], f32, kind="ExternalInput").ap()
    b1 = nc.dram_tensor("b1", [128, GN], f32, kind="ExternalInput").ap()
    wt = nc.dram_tensor("wt", [C, C], f32, kind="ExternalInput").ap()
    bbc = nc.dram_tensor("bbc", [128, C], f32, kind="ExternalInput").ap()
    # y ships int8 with a per-node scale (relu output, rowmax/127 quant):
    # halves the slow D2H vs bf16 at ~0.4%-of-max worst-case error.
    y = nc.dram_tensor("y", [NPC, C], mybir.dt.int8, kind="ExternalOutput").ap()
    sc = nc.dram_tensor("sc", [NPC, 1], f32, kind="ExternalOutput").ap()

    NR = TOTN // 128  # 256 table ranks
    NW = NTOK // 16   # 264 wrapped index columns

    with tile.TileContext(nc) as tc:
        nc.gpsimd.load_library(library_config.mlp)
        with (
            tc.tile_pool(name="const", bufs=1) as cpool,
            tc.tile_pool(name="gath", bufs=2) as gpool,
            tc.tile_pool(name="work", bufs=3) as wpool,
            tc.tile_pool(name="tiny", bufs=4) as tpool,
            tc.tile_pool(name="psS", bufs=2, space="PSUM") as psS,
            tc.tile_pool(name="psW", bufs=2, space="PSUM") as psW,
            tc.tile_pool(name="psZ", bufs=2, space="PSUM") as psZ,
            tc.tile_pool(name="psY", bufs=2, space="PSUM") as psY,
        ):
            # ---- persistent constants / tables ----
            table = cpool.tile([128, NR, C], bf16, tag="table")
            xtab_v = xtab.rearrange("(r t) c -> t r c", t=128)
            for rb in range(0, NR, NR // 8):
                nc.gpsimd.dma_start(
                    out=table[:, rb : rb + NR // 8, :],
                    in_=xtab_v[:, rb : rb + NR // 8, :],
                )
            # indices: broadcast 16-partition wrapped layout to all 128
            idx_sb = cpool.tile([128, NCH, NW], i16, tag="idx")
            for rep in range(8):
                nc.sync.dma_start(out=idx_sb[16 * rep : 16 * (rep + 1), :, :], in_=idxw)
            mask_sb = cpool.tile([128, 128], f32, tag="mask")
            nc.sync.dma_start(out=mask_sb[:], in_=maskneg)
            b1_sb = cpool.tile([128, GN], f32, tag="b1")
            nc.sync.dma_start(out=b1_sb[:], in_=b1)
            wt_sb = cpool.tile([C, C], f32, tag="wt")
            nc.sync.dma_start(out=wt_sb[:], in_=wt)
            bbc_sb = cpool.tile([128, C], f32, tag="bbc")
            nc.sync.dma_start(out=bbc_sb[:], in_=bbc)

            table_raw = table[:].rearrange("p r c -> p (r c)")

            GSZ = 896  # max idxs per dma_gather instruction (1024 crashes HW)

            def _chunks(total):
                o = 0
                while o < total:
                    n = min(GSZ, total - o)
                    yield o, n
                    o += n

            for ch in range(NCH):
                # gathered bf16 columns: [:, :2048]=XI, [:, 2048:4096]=XJ,
                # [:, 4096:4224]=OWN (residual x for this chunk's nodes)
                cols = gpool.tile([128, 1, NTOK], bf16, tag="cols")
                for o, n in _chunks(NTOK):
                    nc.gpsimd.dma_gather(
                        out_ap=cols[:, :, o : o + n],
                        in_ap=table_raw,
                        idxs_ap=idx_sb[:, ch, o // 16 : (o + n) // 16],
                        num_idxs=n,
                        num_idxs_reg=n,
                        elem_size=C,
                        transpose=True,
                        sbuf_tokens_per_rank=128,
                        sbuf_free_dim_per_rank=2 * C,
                    )
                colsv = cols[:].rearrange("p one n -> p (one n)")
                # gathered bf16 rows of x_j (XJ index block): [128=(m,j), g, c]
                xjr = gpool.tile([128, G, C], bf16, tag="xjr")
                for o, n in _chunks(CHUNK * G):
                    nc.gpsimd.dma_gather(
                        out_ap=xjr[:, o // 128 : (o + n) // 128, :],
                        in_ap=xtab,
                        idxs_ap=idx_sb[:, ch, 128 + o // 16 : 128 + (o + n) // 16],
                        num_idxs=n,
                        num_idxs_reg=n,
                        elem_size=C,
                    )

                zps = psZ.tile([128, CHUNK], f32, tag="zps")
                for g in range(G):
                    ps = psS.tile([128, 128], f32, tag="ps")
                    nc.tensor.matmul(
                        ps[:],
                        lhsT=colsv[:, g * 128 : (g + 1) * 128],
                        rhs=colsv[:, 2048 + g * 128 : 2048 + (g + 1) * 128],
                        start=True,
                        stop=True,
                    )
                    ms = wpool.tile([128, 128], f32, tag="ms")
                    nc.vector.tensor_add(ms[:], ps[:], mask_sb[:])
                    E = wpool.tile([128, 128], bf16, tag="E")
                    Z = tpool.tile([128, 1], f32, tag="Z")
                    nc.scalar.activation(
                        E[:], ms[:], mybir.ActivationFunctionType.Exp,
                        scale=SCALE, accum_out=Z[:],
                    )
                    R = tpool.tile([128, 1], f32, tag="R")
                    nc.vector.reciprocal(R[:], Z[:])
                    b1r = tpool.tile([128, GN], bf16, tag="b1r")
                    nc.vector.tensor_scalar_mul(b1r[:], b1_sb[:], R[:])
                    pw = psW.tile([128, GN], f32, tag="pw")
                    nc.tensor.matmul(pw[:], lhsT=E[:], rhs=b1r[:], start=True, stop=True)
                    wm = tpool.tile([128, GN], bf16, tag="wm")
                    nc.vector.tensor_copy(wm[:], pw[:])
                    nc.tensor.matmul(
                        zps[:, g * GN : (g + 1) * GN],
                        lhsT=xjr[:, g, :],
                        rhs=wm[:],
                        start=True,
                        stop=True,
                    )

                # z = x_own^T + h^T : OWN cols block is the residual
                ownf = wpool.tile([128, CHUNK], f32, tag="ownf")
                nc.vector.tensor_copy(ownf[:], colsv[:, 4096:4224])
                zsb = wpool.tile([128, CHUNK], f32, tag="zsb")
                nc.vector.tensor_add(zsb[:], zps[:], ownf[:])
                yps = psY.tile([128, C], f32, tag="yps")
                nc.tensor.matmul(yps[:], lhsT=zsb[:], rhs=wt_sb[:], start=True, stop=True)
                ysb = wpool.tile([128, C], f32, tag="ysb")
                nc.vector.tensor_add(ysb[:], yps[:], bbc_sb[:])
                yr = wpool.tile([128, C], f32, tag="yr")
                nc.scalar.activation(yr[:], ysb[:], mybir.ActivationFunctionType.Relu)
                rmax = tpool.tile([128, 1], f32, tag="rmax")
                nc.vector.tensor_reduce(
                    rmax[:], yr[:], mybir.AxisListType.X, mybir.AluOpType.max
                )
                rsc = tpool.tile([128, 1], f32, tag="rsc")
                nc.vector.tensor_scalar(
                    rsc[:], rmax[:], 1.0 / 127.0, 1e-30,
                    mybir.AluOpType.mult, mybir.AluOpType.max,
                )
                rs = tpool.tile([128, 1], f32, tag="rs")
                nc.vector.reciprocal(rs[:], rsc[:])
                yq = wpool.tile([128, C], mybir.dt.int8, tag="yq")
                nc.vector.tensor_scalar_mul(yq[:], yr[:], rs[:])
                nc.sync.dma_start(out=y[ch * 128 : (ch + 1) * 128, :], in_=yq[:])
                nc.sync.dma_start(out=sc[ch * 128 : (ch + 1) * 128, :], in_=rsc[:])
    nc.compile()
    return nc


PROWS = TOTN + C + 1 + 7  # packed H2D stream: x rows, W rows, b row, pad to /8


def make_idx(edge_index):
    """Global wrapped index tensor [8*16, NCH, 264] i16.

    Per core: tokens per chunk of 128 nodes = [e1(2048) | e0(2048) | own(128)],
    each +16384 for batch-1 cores (fused two-batch table), wrapped so token t
    sits at (partition t%16, column t//16). Core c = batch c//4, slice c%4,
    which is exactly row-major order of the [2, 4, ...] reshape."""
    e = np.asarray(edge_index)
    offs = (np.arange(CORES, dtype=e.dtype) // 4 * N)[:, None, None]
    e1 = e[1].reshape(CORES, NCH, CHUNK * G) + offs
    e0 = e[0].reshape(CORES, NCH, CHUNK * G) + offs
    own = np.broadcast_to(
        np.arange(TOTN, dtype=e.dtype).reshape(CORES, NCH, CHUNK), e1[..., :CHUNK].shape
    )
    a = np.concatenate([e1, e0, own], axis=2).astype(np.int16)  # [8, NCH, 4224]
    w = a.reshape(CORES, NCH, NTOK // 16, 16).transpose(0, 3, 1, 2)
    return np.ascontiguousarray(w.reshape(CORES * 16, NCH, NTOK // 16))


_CACHE = {}


def _setup():
    bass2jax.install_neuronx_cc_hook()
    nc = build_nc()
    assert nc.dbg_addr is None
    devs = jax.devices()[:CORES]
    mesh = Mesh(np.asarray(devs), ("core",))

    in_names, out_names, out_avals = [], [], []
    for alloc in nc.m.functions[0].allocations:
        if not isinstance(alloc, mybir.MemoryLocationSet):
            continue
        name = alloc.memorylocations[0].name
        if alloc.kind == "ExternalInput":
            if nc.partition_id_tensor is None or name != nc.partition_id_tensor.name:
                in_names.append(name)
        elif alloc.kind == "ExternalOutput":
            out_names.append(name)
            out_avals.append(
                jax.core.ShapedArray(tuple(alloc.tensor_shape), mybir.dt.np(alloc.dtype))
            )
    n_params, n_outs = len(in_names), len(out_names)
    pname = nc.partition_id_tensor.name if nc.partition_id_tensor else None
    all_in = tuple(in_names) + tuple(out_names) + ((pname,) if pname else ())

    def _body(*args):
        operands = list(args)
        if pname is not None:
            operands.append(bass2jax.partition_id_tensor())
        outs = bass2jax._bass_exec_p.bind(
            *operands,
            out_avals=tuple(out_avals),
            in_names=all_in,
            out_names=tuple(out_names),
            lowering_input_output_aliases=(),
            sim_require_finite=True,
            sim_require_nnan=True,
            nc=nc,
        )
        return tuple(outs)

    donate = tuple(range(n_params, n_params + n_outs))
    run = jax.jit(
        shard_map(
            _body, mesh=mesh,
            in_specs=(P("core"),) * (n_params + n_outs),
            out_specs=(P("core"),) * n_outs,
            check_rep=False,
        ),
        donate_argnums=donate,
        keep_unused=True,
    )

    def _prep(xsh):
        full = jax.lax.all_gather(xsh, "core", axis=0, tiled=True)  # [PROWS,128] bf16
        tab = full[:TOTN]
        wt = full[TOTN : TOTN + C].astype(jnp.float32).T
        bbc = jnp.tile(full[TOTN + C].astype(jnp.float32)[None, :], (128, 1))
        y0 = jnp.zeros((NPC, C), jnp.int8)
        s0 = jnp.zeros((NPC, 1), jnp.float32)
        return tab, y0, s0, wt, bbc

    prep = jax.jit(
        shard_map(
            _prep, mesh=mesh,
            in_specs=(P("core"),),
            out_specs=(P("core"),) * 5,
            check_rep=False,
        )
    )

    def _consts():
        i = jnp.arange(128)
        mask = jnp.where(
            (i[:, None] // K) == (i[None, :] // K), 0.0, -1e9
        ).astype(jnp.float32)
        b1m = ((i[:, None] // K) == jnp.arange(GN)[None, :]).astype(jnp.float32)
        return mask, b1m

    constF = jax.jit(
        shard_map(
            _consts, mesh=mesh, in_specs=(), out_specs=(P("core"),) * 2,
            check_rep=False,
        )
    )
    maskD, b1D = constF()
    _CACHE.update(
        nc=nc, run=run, prep=prep, in_names=in_names, out_names=out_names,
        maskD=maskD, b1D=b1D,
    )


def kernel(x, edge_index, W, b, **kw):
    if "run" not in _CACHE:
        _setup()
    xb = np.asarray(x, dtype=np.float32).astype(ml_dtypes.bfloat16).reshape(TOTN, C)
    Wb = np.asarray(W, dtype=np.float32).astype(ml_dtypes.bfloat16)
    bb = np.asarray(b, dtype=np.float32).astype(ml_dtypes.bfloat16)[None, :]
    pad = np.zeros((PROWS - TOTN - C - 1, C), ml_dtypes.bfloat16)
    xg = np.concatenate([xb, Wb, bb, pad], axis=0)
    tabD, y0D, s0D, wtD, bbcD = _CACHE["prep"](xg)
    idxg = make_idx(edge_index)
    args = {"xtab": tabD, "idxw": idxg, "maskneg": _CACHE["maskD"],
            "b1": _CACHE["b1D"], "wt": wtD, "bbc": bbcD}
    zeros = {"y": y0D, "sc": s0D}
    outs = _CACHE["run"](
        *[args[n] for n in _CACHE["in_names"]],
        *[zeros[n] for n in _CACHE["out_names"]],
    )
    out_by_name = dict(zip(_CACHE["out_names"], outs))
    yD, scD = out_by_name["y"], out_by_name["sc"]
    for o in (yD, scD):
        try:
            o.copy_to_host_async()
        except Exception:
            pass
    y = np.empty((B, N, C), np.float32)
    yv = y.reshape(CORES, NPC, C)

    def _fetch(i, shy, shs):
        yv[i] = np.asarray(shy.data).astype(np.float32) * np.asarray(shs.data)

    try:
        ysh = sorted(yD.addressable_shards, key=lambda s: s.index[0].start or 0)
        ssh = sorted(scD.addressable_shards, key=lambda s: s.index[0].start or 0)
        assert len(ysh) == CORES and len(ssh) == CORES
        with cf.ThreadPoolExecutor(CORES) as ex:
            list(ex.map(lambda t: _fetch(*t), zip(range(CORES), ysh, ssh)))
    except Exception:
        y = (
            np.asarray(yD).astype(np.float32) * np.asarray(scD)
        ).reshape(B, N, C)
    return y.reshape(B, N, C)


# revision 19
# speedup vs baseline: 1.4102x; 1.4102x over previous
"""Trainium2 Bass kernel for nn_AttenConv1d (GNN message passing attention).

Per node n (batch b):
  x_i = x[b, idx1[n,:]]   [16,128]   (centers)
  x_j = x[b, idx0[n,:]]   [16,128]   (neighbors)
  S = x_i @ x_j.T / sqrt(128)        [16,16]
  P = softmax(S, -1)
  h = (P @ x_j).sum(0)               [128]
  y = relu((x[b,n] + h) @ W.T + b)

8 cores: core c handles batch c//4, node slice (c%4)*4096. This problem is
tunnel-transfer-bound (axon H2D ~68MB/s, D2H ~29MB/s), so the pipeline is
built to minimize host<->device bytes:
  - x ships once as bf16 node shards (1MB/core); a jax prelude all-gathers
    the full 32768-row two-batch table on-device and feeds it directly as
    the bass kernel's table parameter (device-resident, no replication over
    the tunnel). mask/b1/W^T/bias-broadcast consts are also built on-device.
  - indices ship un-replicated as [16, nch, 264] i16 and are broadcast to
    the 128-partition wrapped layout dma_gather wants with 8 on-device DMAs.
    Layout per chunk of 128 nodes: [XI 2048 | XJ 2048 | OWN 128] tokens;
    the OWN block doubles as the residual (replaces a PE transpose of a
    separately-shipped xown tensor), and the XJ block doubles as the row
    gather index list (replaces a separate idxr tensor).
  - y returns as bf16 (halves the slow D2H) and is upcast on host.
On-chip per core: bf16 table [128, 256, 128] in SBUF; dma_gather(transpose)
for score columns, DRAM row gather for values; groups of 8 nodes = 128
(node,k) pairs fill the partition dim; block-diagonal bf16 score matmul,
masked exp softmax with fused row-sum, two small matmuls per group, fused
final linear.
"""

import concurrent.futures as cf
import math
import sys

import numpy as np

for _p in ("/opt/trn_rl_repo",):
    if _p not in sys.path:
        sys.path.insert(0, _p)

import jax
import jax.numpy as jnp
import ml_dtypes
from jax.sharding import Mesh, PartitionSpec as P

try:
    from jax.experimental.shard_map import shard_map
except ImportError:
    from jax.shard_map import shard_map

import concourse.bass as bass
import concourse.bacc as bacc
import concourse.mybir as mybir
from concourse import bass2jax, library_config, tile

B, N, K, C = 2, 16384, 128 // 8, 128  # K=16
CORES = 8
TOTN = B * N                  # 32768 rows in the fused two-batch table
NPC = TOTN // CORES           # nodes per core = 4096
CHUNK = 128                   # nodes per chunk
NCH = NPC // CHUNK            # chunks per core = 32
G = 16                        # groups per chunk (8 nodes each)
GN = CHUNK // G               # nodes per group = 8
NTOK = 2 * CHUNK * G + CHUNK  # gathered col tokens per chunk = 4224
SCALE = 1.0 / math.sqrt(C)

f32 = mybir.dt.float32
bf16 = mybir.dt.bfloat16
i16 = mybir.dt.int16


def build_nc():
    nc = bacc.Bacc("TRN2", target_bir_lowering=False, debug=False)
    xtab = nc.dram_tensor("xtab", [TOTN, C], bf16, kind="ExternalInput").ap()
    idxw = nc.dram_tensor("idxw", [16, NCH, NTOK // 16], i16, kind="ExternalInput").ap()
    maskneg = nc.dram_tensor("maskneg", [128, 128], f32, kind="ExternalInput").ap()
    b1 = nc.dram_tensor("b1", [128, GN], f32, kind="ExternalInput").ap()
    wt = nc.dram_tensor("wt", [C, C], f32, kind="ExternalInput").ap()
    bbc = nc.dram_tensor("bbc", [128, C], f32, kind="ExternalInput").ap()
    # y ships int8 with a per-node scale (relu output, rowmax/127 quant):
    # halves the slow D2H vs bf16 at ~0.4%-of-max worst-case error. The f32
    # scale rides in the last 4 columns so the fetch is a single array.
    y = nc.dram_tensor("y", [NPC, C + 4], mybir.dt.int8, kind="ExternalOutput").ap()

    NR = TOTN // 128  # 256 table ranks
    NW = NTOK // 16   # 264 wrapped index columns

    with tile.TileContext(nc) as tc:
        nc.gpsimd.load_library(library_config.mlp)
        with (
            tc.tile_pool(name="const", bufs=1) as cpool,
            tc.tile_pool(name="gath", bufs=2) as gpool,
            tc.tile_pool(name="work", bufs=3) as wpool,
            tc.tile_pool(name="tiny", bufs=4) as tpool,
            tc.tile_pool(name="psS", bufs=2, space="PSUM") as psS,
            tc.tile_pool(name="psW", bufs=2, space="PSUM") as psW,
            tc.tile_pool(name="psZ", bufs=2, space="PSUM") as psZ,
            tc.tile_pool(name="psY", bufs=2, space="PSUM") as psY,
        ):
            # ---- persistent constants / tables ----
            table = cpool.tile([128, NR, C], bf16, tag="table")
            xtab_v = xtab.rearrange("(r t) c -> t r c", t=128)
            for rb in range(0, NR, NR // 8):
                nc.gpsimd.dma_start(
                    out=table[:, rb : rb + NR // 8, :],
                    in_=xtab_v[:, rb : rb + NR // 8, :],
                )
            # indices: broadcast 16-partition wrapped layout to all 128
            idx_sb = cpool.tile([128, NCH, NW], i16, tag="idx")
            for rep in range(8):
                nc.sync.dma_start(out=idx_sb[16 * rep : 16 * (rep + 1), :, :], in_=idxw)
            mask_sb = cpool.tile([128, 128], f32, tag="mask")
            nc.sync.dma_start(out=mask_sb[:], in_=maskneg)
            b1_sb = cpool.tile([128, GN], f32, tag="b1")
            nc.sync.dma_start(out=b1_sb[:], in_=b1)
            wt_sb = cpool.tile([C, C], f32, tag="wt")
            nc.sync.dma_start(out=wt_sb[:], in_=wt)
            bbc_sb = cpool.tile([128, C], f32, tag="bbc")
            nc.sync.dma_start(out=bbc_sb[:], in_=bbc)

            table_raw = table[:].rearrange("p r c -> p (r c)")

            GSZ = 896  # max idxs per dma_gather instruction (1024 crashes HW)

            def _chunks(total):
                o = 0
                while o < total:
                    n = min(GSZ, total - o)
                    yield o, n
                    o += n

            for ch in range(NCH):
                # gathered bf16 columns: [:, :2048]=XI, [:, 2048:4096]=XJ,
                # [:, 4096:4224]=OWN (residual x for this chunk's nodes)
                cols = gpool.tile([128, 1, NTOK], bf16, tag="cols")
                for o, n in _chunks(NTOK):
                    nc.gpsimd.dma_gather(
                        out_ap=cols[:, :, o : o + n],
                        in_ap=table_raw,
                        idxs_ap=idx_sb[:, ch, o // 16 : (o + n) // 16],
                        num_idxs=n,
                        num_idxs_reg=n,
                        elem_size=C,
                        transpose=True,
                        sbuf_tokens_per_rank=128,
                        sbuf_free_dim_per_rank=2 * C,
                    )
                colsv = cols[:].rearrange("p one n -> p (one n)")
                # gathered bf16 rows of x_j (XJ index block): [128=(m,j), g, c]
                xjr = gpool.tile([128, G, C], bf16, tag="xjr")
                for o, n in _chunks(CHUNK * G):
                    nc.gpsimd.dma_gather(
                        out_ap=xjr[:, o // 128 : (o + n) // 128, :],
                        in_ap=xtab,
                        idxs_ap=idx_sb[:, ch, 128 + o // 16 : 128 + (o + n) // 16],
                        num_idxs=n,
                        num_idxs_reg=n,
                        elem_size=C,
                    )

                zps = psZ.tile([128, CHUNK], f32, tag="zps")
                for g in range(G):
                    ps = psS.tile([128, 128], f32, tag="ps")
                    nc.tensor.matmul(
                        ps[:],
                        lhsT=colsv[:, g * 128 : (g + 1) * 128],
                        rhs=colsv[:, 2048 + g * 128 : 2048 + (g + 1) * 128],
                        start=True,
                        stop=True,
                    )
                    ms = wpool.tile([128, 128], f32, tag="ms")
                    nc.vector.tensor_add(ms[:], ps[:], mask_sb[:])
                    E = wpool.tile([128, 128], bf16, tag="E")
                    Z = tpool.tile([128, 1], f32, tag="Z")
                    nc.scalar.activation(
                        E[:], ms[:], mybir.ActivationFunctionType.Exp,
                        scale=SCALE, accum_out=Z[:],
                    )
                    R = tpool.tile([128, 1], f32, tag="R")
                    nc.vector.reciprocal(R[:], Z[:])
                    b1r = tpool.tile([128, GN], bf16, tag="b1r")
                    nc.vector.tensor_scalar_mul(b1r[:], b1_sb[:], R[:])
                    pw = psW.tile([128, GN], f32, tag="pw")
                    nc.tensor.matmul(pw[:], lhsT=E[:], rhs=b1r[:], start=True, stop=True)
                    wm = tpool.tile([128, GN], bf16, tag="wm")
                    nc.vector.tensor_copy(wm[:], pw[:])
                    nc.tensor.matmul(
                        zps[:, g * GN : (g + 1) * GN],
                        lhsT=xjr[:, g, :],
                        rhs=wm[:],
                        start=True,
                        stop=True,
                    )

                # z = x_own^T + h^T : OWN cols block is the residual
                ownf = wpool.tile([128, CHUNK], f32, tag="ownf")
                nc.vector.tensor_copy(ownf[:], colsv[:, 4096:4224])
                zsb = wpool.tile([128, CHUNK], f32, tag="zsb")
                nc.vector.tensor_add(zsb[:], zps[:], ownf[:])
                yps = psY.tile([128, C], f32, tag="yps")
                nc.tensor.matmul(yps[:], lhsT=zsb[:], rhs=wt_sb[:], start=True, stop=True)
                ysb = wpool.tile([128, C], f32, tag="ysb")
                nc.vector.tensor_add(ysb[:], yps[:], bbc_sb[:])
                yr = wpool.tile([128, C], f32, tag="yr")
                nc.scalar.activation(yr[:], ysb[:], mybir.ActivationFunctionType.Relu)
                rmax = tpool.tile([128, 1], f32, tag="rmax")
                nc.vector.tensor_reduce(
                    rmax[:], yr[:], mybir.AxisListType.X, mybir.AluOpType.max
                )
                rsc = tpool.tile([128, 1], f32, tag="rsc")
                nc.vector.tensor_scalar(
                    rsc[:], rmax[:], 1.0 / 127.0, 1e-30,
                    mybir.AluOpType.mult, mybir.AluOpType.max,
                )
                rs = tpool.tile([128, 1], f32, tag="rs")
                nc.vector.reciprocal(rs[:], rsc[:])
                yq = wpool.tile([128, C + 4], mybir.dt.int8, tag="yq")
                nc.vector.tensor_scalar_mul(yq[:, :C], yr[:], rs[:])
                nc.vector.tensor_copy(yq[:, C:], rsc[:].bitcast(mybir.dt.int8))
                nc.sync.dma_start(out=y[ch * 128 : (ch + 1) * 128, :], in_=yq[:])
    nc.compile()
    return nc


PXI = TOTN + C + 1 + 7        # packed H2D stream: x rows, W rows, b row, pad
IDXROWS = CORES * 16 * NCH * (NTOK // 16) // 128  # idx bytes as bf16 rows: 8448
PROWS = PXI + IDXROWS         # 41352, /8 per-core shards


def make_idx(edge_index):
    """Global wrapped index tensor [8*16, NCH, 264] i16.

    Per core: tokens per chunk of 128 nodes = [e1(2048) | e0(2048) | own(128)],
    each +16384 for batch-1 cores (fused two-batch table), wrapped so token t
    sits at (partition t%16, column t//16). Core c = batch c//4, slice c%4,
    which is exactly row-major order of the [2, 4, ...] reshape."""
    e = np.asarray(edge_index)
    offs = (np.arange(CORES, dtype=e.dtype) // 4 * N)[:, None, None]
    e1 = e[1].reshape(CORES, NCH, CHUNK * G) + offs
    e0 = e[0].reshape(CORES, NCH, CHUNK * G) + offs
    own = np.broadcast_to(
        np.arange(TOTN, dtype=e.dtype).reshape(CORES, NCH, CHUNK), e1[..., :CHUNK].shape
    )
    a = np.concatenate([e1, e0, own], axis=2).astype(np.int16)  # [8, NCH, 4224]
    w = a.reshape(CORES, NCH, NTOK // 16, 16).transpose(0, 3, 1, 2)
    return np.ascontiguousarray(w.reshape(CORES * 16, NCH, NTOK // 16))


_CACHE = {}


def _setup():
    bass2jax.install_neuronx_cc_hook()
    nc = build_nc()
    assert nc.dbg_addr is None
    devs = jax.devices()[:CORES]
    mesh = Mesh(np.asarray(devs), ("core",))

    in_names, out_names, out_avals = [], [], []
    for alloc in nc.m.functions[0].allocations:
        if not isinstance(alloc, mybir.MemoryLocationSet):
            continue
        name = alloc.memorylocations[0].name
        if alloc.kind == "ExternalInput":
            if nc.partition_id_tensor is None or name != nc.partition_id_tensor.name:
                in_names.append(name)
        elif alloc.kind == "ExternalOutput":
            out_names.append(name)
            out_avals.append(
                jax.core.ShapedArray(tuple(alloc.tensor_shape), mybir.dt.np(alloc.dtype))
            )
    n_params, n_outs = len(in_names), len(out_names)
    pname = nc.partition_id_tensor.name if nc.partition_id_tensor else None
    all_in = tuple(in_names) + tuple(out_names) + ((pname,) if pname else ())

    def _body(*args):
        operands = list(args)
        if pname is not None:
            operands.append(bass2jax.partition_id_tensor())
        outs = bass2jax._bass_exec_p.bind(
            *operands,
            out_avals=tuple(out_avals),
            in_names=all_in,
            out_names=tuple(out_names),
            lowering_input_output_aliases=(),
            sim_require_finite=True,
            sim_require_nnan=True,
            nc=nc,
        )
        return tuple(outs)

    donate = tuple(range(n_params, n_params + n_outs))
    run = jax.jit(
        shard_map(
            _body, mesh=mesh,
            in_specs=(P("core"),) * (n_params + n_outs),
            out_specs=(P("core"),) * n_outs,
            check_rep=False,
        ),
        donate_argnums=donate,
        keep_unused=True,
    )

    def _prep(xsh):
        # int16 transport: idx bits must not pass through a float dtype
        # (bf16 NaN payloads get canonicalized in transit, corrupting
        # indices 32640..32767). x/W/b rows are bitcast back to bf16 here.
        fulli = jax.lax.all_gather(xsh, "core", axis=0, tiled=True)  # [PROWS,128] i16
        fullf = jax.lax.bitcast_convert_type(fulli[:PXI], jnp.bfloat16)
        tab = fullf[:TOTN]
        wt = fullf[TOTN : TOTN + C].astype(jnp.float32).T
        bbc = jnp.tile(fullf[TOTN + C].astype(jnp.float32)[None, :], (128, 1))
        idxb = fulli[PXI:]  # [8448,128] i16
        c = jax.lax.axis_index("core")
        rows = IDXROWS // CORES
        myidx = jax.lax.dynamic_slice(
            idxb, (c * rows, 0), (rows, 128)
        ).reshape(16, NCH, NTOK // 16)
        y0 = jnp.zeros((NPC, C + 4), jnp.int8)
        return tab, myidx, y0, wt, bbc

    prep = jax.jit(
        shard_map(
            _prep, mesh=mesh,
            in_specs=(P("core"),),
            out_specs=(P("core"),) * 5,
            check_rep=False,
        )
    )

    def _consts():
        i = jnp.arange(128)
        mask = jnp.where(
            (i[:, None] // K) == (i[None, :] // K), 0.0, -1e9
        ).astype(jnp.float32)
        b1m = ((i[:, None] // K) == jnp.arange(GN)[None, :]).astype(jnp.float32)
        return mask, b1m

    constF = jax.jit(
        shard_map(
            _consts, mesh=mesh, in_specs=(), out_specs=(P("core"),) * 2,
            check_rep=False,
        )
    )
    maskD, b1D = constF()
    _CACHE.update(
        nc=nc, run=run, prep=prep, in_names=in_names, out_names=out_names,
        maskD=maskD, b1D=b1D,
    )


def kernel(x, edge_index, W, b, **kw):
    if "run" not in _CACHE:
        _setup()
    xb = np.asarray(x, dtype=np.float32).astype(ml_dtypes.bfloat16).reshape(TOTN, C)
    Wb = np.asarray(W, dtype=np.float32).astype(ml_dtypes.bfloat16)
    bb = np.asarray(b, dtype=np.float32).astype(ml_dtypes.bfloat16)[None, :]
    pad = np.zeros((PXI - TOTN - C - 1, C), ml_dtypes.bfloat16)
    idxg = make_idx(edge_index)
    idxrows = idxg.reshape(IDXROWS, 128).view(ml_dtypes.bfloat16)
    xg = np.concatenate([xb, Wb, bb, pad, idxrows], axis=0).view(np.int16)
    tabD, idxD, y0D, wtD, bbcD = _CACHE["prep"](xg)
    args = {"xtab": tabD, "idxw": idxD, "maskneg": _CACHE["maskD"],
            "b1": _CACHE["b1D"], "wt": wtD, "bbc": bbcD}
    outs = _CACHE["run"](*[args[n] for n in _CACHE["in_names"]], y0D)
    yD = outs[0]
    try:
        yD.copy_to_host_async()
    except Exception:
        pass
    y = np.empty((B, N, C), np.float32)
    yv = y.reshape(CORES, NPC, C)

    def _fetch(i, shy):
        q = np.asarray(shy.data)  # [NPC, C+4] int8
        s = np.ascontiguousarray(q[:, C:]).view(np.float32)  # [NPC, 1]
        yv[i] = q[:, :C].astype(np.float32) * s

    try:
        ysh = sorted(yD.addressable_shards, key=lambda s: s.index[0].start or 0)
        assert len(ysh) == CORES
        with cf.ThreadPoolExecutor(CORES) as ex:
            list(ex.map(lambda t: _fetch(*t), enumerate(ysh)))
    except Exception:
        q = np.asarray(yD).reshape(CORES, NPC, C + 4)
        s = np.ascontiguousarray(q[:, :, C:]).view(np.float32).reshape(CORES, NPC, 1)
        yv[:] = q[:, :, :C].astype(np.float32) * s
    return y.reshape(B, N, C)
